# revision 3
# baseline (speedup 1.0000x reference)
"""Trainium2 Bass kernel for the MoE routing layer (nn_MoELayer_20358144983731).

Strategy
--------
Routing depends only on the atom's type (32 types), and with top-2-of-8
routing each atom needs exactly 3 expert MLPs (2 routed + 1 shared) instead
of the reference's dense 9.  The gate is tiny, so it is computed on the host;
atoms are sorted by type, each type block padded to a fixed length L, and the
32 type blocks are distributed 4-per-core across the 8 NeuronCores.  The per
type routing weights (w0, w1) are scalars shared by every atom of the type,
so the whole device program is data-driven (weights / biases / scales arrive
as per-core input tensors) and a single SPMD program runs on all 8 cores.

Per core (g = 4 type groups):
    y[g] = w0*tanh(X W0 + b0) + w1*tanh(X W1 + b1) + tanh(X Ws + bs)
computed transposed (z.T = [dout, atoms]) so the dout-dim bias lands on
partitions (free in ACT) and matmuls keep W stationary.  Matmuls run in
bf16 with fp32 PSUM accumulation; tanh on the scalar engine; the 3-stream
combine is two fused scalar_tensor_tensor ops on the vector engine.
"""

import sys

import numpy as np

try:
    import concourse  # noqa: F401
except ImportError:  # grading container path
    sys.path.insert(0, "/opt/trn_rl_repo")

import ml_dtypes

import concourse.bacc as bacc
import concourse.mybir as mybir
import concourse.tile as tile
from concourse.bass_utils import run_bass_kernel_spmd

NB, NLOC = 4, 16384
DIN, DOUT = 256, 256
NTYPES = 32
N_CORES = 8
G = NTYPES // N_CORES  # type groups per core = 4
NS = 3  # streams: routed expert 0, routed expert 1, shared expert
BF16 = ml_dtypes.bfloat16

_compiled_cache = {}


def _build_nc(L):
    """Build + compile the SPMD Tile kernel for per-type capacity L."""
    f32 = mybir.dt.float32
    bf16 = mybir.dt.bfloat16
    Tanh = mybir.ActivationFunctionType.Tanh
    mult = mybir.AluOpType.mult
    add = mybir.AluOpType.add

    # atom chunks: bank-aligned (multiples of 512) pieces up to 2048
    chunks = []
    off = 0
    while off < L:
        cl = min(2048, L - off)
        chunks.append((off, cl))
        off += cl

    nc = bacc.Bacc("TRN2", target_bir_lowering=False, debug=False)
    xt_d = nc.dram_tensor("xt", [G * 128, 2 * L], bf16, kind="ExternalInput")
    w_d = nc.dram_tensor("w", [128, G * NS * 2 * 2 * 128], bf16, kind="ExternalInput")
    b_d = nc.dram_tensor("b", [128, G * NS * 2], f32, kind="ExternalInput")
    s_d = nc.dram_tensor("s", [128, G * 2], f32, kind="ExternalInput")
    y_d = nc.dram_tensor("y", [G * 2 * 128, L], f32, kind="ExternalOutput")

    with tile.TileContext(nc) as tc:
        with (
            tc.tile_pool(name="const", bufs=1) as constp,
            tc.tile_pool(name="xt", bufs=2) as xtp,
            tc.tile_pool(name="t", bufs=2) as tp,
            tc.tile_pool(name="y", bufs=2) as yp,
            tc.tile_pool(name="ps", bufs=2, space="PSUM") as psp,
        ):
            w_sb = constp.tile([128, G * NS * 2 * 2 * 128], bf16)
            nc.sync.dma_start(out=w_sb, in_=w_d.ap())
            b_sb = constp.tile([128, G * NS * 2], f32)
            nc.sync.dma_start(out=b_sb, in_=b_d.ap())
            s_sb = constp.tile([128, G * 2], f32)
            nc.sync.dma_start(out=s_sb, in_=s_d.ap())

            for g in range(G):
                xt_sb = xtp.tile([128, 2 * L], bf16, tag="xt")
                nc.sync.dma_start(
                    out=xt_sb, in_=xt_d.ap()[g * 128 : (g + 1) * 128]
                )
                for c in range(2):
                    t_sb = [
                        tp.tile([128, L], f32, tag=f"t{s}", name=f"t{s}")
                        for s in range(NS)
                    ]
                    for s in range(NS):
                        bcol = (g * NS + s) * 2 + c
                        for (c0, cl) in chunks:
                            ps = psp.tile([128, cl], f32, tag="ps")
                            for k in range(2):
                                blk = bcol * 2 + k
                                lhsT = w_sb[:, blk * 128 : (blk + 1) * 128]
                                for a0 in range(0, cl, 512):
                                    al = min(512, cl - a0)
                                    nc.tensor.matmul(
                                        ps[:, a0 : a0 + al],
                                        lhsT,
                                        xt_sb[:, k * L + c0 + a0 : k * L + c0 + a0 + al],
                                        start=(k == 0),
                                        stop=(k == 1),
                                    )
                            nc.scalar.activation(
                                t_sb[s][:, c0 : c0 + cl],
                                ps,
                                Tanh,
                                bias=b_sb[:, bcol : bcol + 1],
                                scale=1.0,
                            )
                    ya = yp.tile([128, L], f32, tag="ya")
                    nc.vector.scalar_tensor_tensor(
                        ya, t_sb[0], s_sb[:, g * 2 : g * 2 + 1], t_sb[2], mult, add
                    )
                    yb = yp.tile([128, L], f32, tag="yb")
                    nc.vector.scalar_tensor_tensor(
                        yb, t_sb[1], s_sb[:, g * 2 + 1 : g * 2 + 2], ya, mult, add
                    )
                    row = (g * 2 + c) * 128
                    nc.sync.dma_start(
                        out=y_d.ap()[row : row + 128], in_=yb
                    )

    nc.compile()
    return nc


def _host_route(type_embeddings, gate_w):
    """Gate on host: per-type top-2 experts + softmax weights (tiny)."""
    logits = type_embeddings.astype(np.float32) @ gate_w.astype(np.float32)
    top2 = np.argsort(-logits, axis=1, kind="stable")[:, :2]
    tv = np.take_along_axis(logits, top2, axis=1)
    e = np.exp(tv - tv.max(axis=1, keepdims=True))
    wts = e / e.sum(axis=1, keepdims=True)
    return top2, wts


def kernel(x, type_embeddings, atom_types, gate_w, expert_w, expert_b,
           shared_w, shared_b, _trace=False, _trace_kwargs=None):
    x = np.asarray(x, dtype=np.float32)
    type_embeddings = np.asarray(type_embeddings, dtype=np.float32)
    atom_types = np.asarray(atom_types)
    gate_w = np.asarray(gate_w, dtype=np.float32)
    expert_w = np.asarray(expert_w, dtype=np.float32)
    expert_b = np.asarray(expert_b, dtype=np.float32)
    shared_w = np.asarray(shared_w, dtype=np.float32)
    shared_b = np.asarray(shared_b, dtype=np.float32)

    top2, wts = _host_route(type_embeddings, gate_w)

    flat_t = atom_types.reshape(-1).astype(np.int64)
    N = flat_t.size
    order = np.argsort(flat_t, kind="stable")
    counts = np.bincount(flat_t, minlength=NTYPES)
    L = max(int(np.ceil(counts.max() / 128) * 128), 512)

    # scatter sorted atoms into per-type padded blocks [NTYPES, L, DIN]
    pos = np.repeat(np.arange(NTYPES, dtype=np.int64) * L, counts)
    pos += np.arange(N, dtype=np.int64) - np.repeat(
        np.concatenate([[0], np.cumsum(counts)[:-1]]), counts
    )
    padded = np.zeros((NTYPES * L, DIN), np.float32)
    padded[pos] = x.reshape(N, DIN)[order]

    # XT layout per core: [G*128, 2*L] bf16 with [g*128+p, k*L+a]
    xt_all = (
        padded.reshape(NTYPES, L, 2, 128)  # [t, a, k, p]
        .transpose(0, 2, 3, 1)  # [t, k, p, a]
        .astype(BF16)
    )

    # per-(type, stream) weight/bias selection
    w_sel = np.empty((NTYPES, NS, DIN, DOUT), np.float32)
    b_sel = np.empty((NTYPES, NS, DOUT), np.float32)
    for t in range(NTYPES):
        w_sel[t, 0] = expert_w[top2[t, 0]]
        w_sel[t, 1] = expert_w[top2[t, 1]]
        w_sel[t, 2] = shared_w[0]
        b_sel[t, 0] = expert_b[top2[t, 0]]
        b_sel[t, 1] = expert_b[top2[t, 1]]
        b_sel[t, 2] = shared_b[0]

    in_maps = []
    for core in range(N_CORES):
        tsl = slice(core * G, (core + 1) * G)
        xt = np.ascontiguousarray(
            xt_all[tsl].transpose(0, 2, 1, 3)  # [g, p, k, a]
        ).reshape(G * 128, 2 * L)

        wb = (
            w_sel[tsl]  # [G, NS, din, dout]
            .reshape(G, NS, 2, 128, 2, 128)  # [g, s, k, p, c, q]
            .transpose(3, 0, 1, 4, 2, 5)  # [p, g, s, c, k, q]
            .reshape(128, G * NS * 2 * 2 * 128)
            .astype(BF16)
        )
        bb = (
            b_sel[tsl]  # [G, NS, dout]
            .reshape(G, NS, 2, 128)  # [g, s, c, p]
            .transpose(3, 0, 1, 2)  # [p, g, s, c]
            .reshape(128, G * NS * 2)
            .astype(np.float32)
        )
        sb_arr = np.broadcast_to(
            wts[tsl].reshape(1, G * 2), (128, G * 2)
        ).astype(np.float32)
        in_maps.append(
            {"xt": np.ascontiguousarray(xt),
             "w": np.ascontiguousarray(wb),
             "b": np.ascontiguousarray(bb),
             "s": np.ascontiguousarray(sb_arr)}
        )

    if L not in _compiled_cache:
        _compiled_cache[L] = _build_nc(L)
    nc = _compiled_cache[L]

    kwargs = {}
    if _trace:
        kwargs["trace"] = True
        kwargs.update(_trace_kwargs or {})
    res = run_bass_kernel_spmd(nc, in_maps, core_ids=list(range(N_CORES)), **kwargs)

    # reassemble: results[core]["y"] is [G*2*128, L] = [g, c, p, a]
    y_all = np.stack([r["y"] for r in res.results])  # [core, G*2*128, L]
    y_all = y_all.reshape(NTYPES, 2, 128, L).transpose(0, 3, 1, 2)  # [t, a, c, p]
    out = np.zeros((N, DOUT), np.float32)
    out[order] = y_all.reshape(NTYPES * L, DOUT)[pos]
    out = out.reshape(NB, NLOC, DOUT)

    if _trace:
        return out, res
    return out


# revision 5
# speedup vs baseline: 1.0493x; 1.0493x over previous
"""Trainium2 Bass kernel for the MoE routing layer (nn_MoELayer_20358144983731).

Strategy
--------
Routing depends only on the atom's type (32 types), and with top-2-of-8
routing each atom needs exactly 3 expert MLPs (2 routed + 1 shared) instead
of the reference's dense 9.  The gate is tiny, so it is computed on the host;
atoms are sorted by type, each type block padded to a fixed length L, and the
32 type blocks are distributed 4-per-core across the 8 NeuronCores.  The per
type routing weights (w0, w1) are scalars shared by every atom of the type,
so the whole device program is data-driven (weights / biases / scales arrive
as per-core input tensors) and a single SPMD program runs on all 8 cores.

Per core (g = 4 type groups):
    y[g] = w0*tanh(X W0 + b0) + w1*tanh(X W1 + b1) + tanh(X Ws + bs)
computed transposed (z.T = [dout, atoms]) so the dout-dim bias lands on
partitions (free in ACT) and matmuls keep W stationary.  Matmuls run in
bf16 with fp32 PSUM accumulation; tanh on the scalar engine; the 3-stream
combine is two fused scalar_tensor_tensor ops on the vector engine.
"""

import sys

import numpy as np

try:
    import concourse  # noqa: F401
except ImportError:  # grading container path
    sys.path.insert(0, "/opt/trn_rl_repo")

import ml_dtypes

import concourse.bacc as bacc
import concourse.mybir as mybir
import concourse.tile as tile
from concourse.bass_utils import run_bass_kernel_spmd

NB, NLOC = 4, 16384
DIN, DOUT = 256, 256
NTYPES = 32
N_CORES = 8
G = NTYPES // N_CORES  # type groups per core = 4
NS = 3  # streams: routed expert 0, routed expert 1, shared expert
BF16 = ml_dtypes.bfloat16

_compiled_cache = {}


def _build_nc(L):
    """Build + compile the SPMD Tile kernel for per-type capacity L."""
    f32 = mybir.dt.float32
    bf16 = mybir.dt.bfloat16
    Tanh = mybir.ActivationFunctionType.Tanh
    mult = mybir.AluOpType.mult
    add = mybir.AluOpType.add

    # atom chunks: bank-aligned pieces up to 1024 (2 PSUM banks each)
    chunks = []
    off = 0
    while off < L:
        cl = min(1024, L - off)
        chunks.append((off, cl))
        off += cl

    nc = bacc.Bacc("TRN2", target_bir_lowering=False, debug=False)
    xt_d = nc.dram_tensor("xt", [G * 128, 2 * L], bf16, kind="ExternalInput")
    w_d = nc.dram_tensor("w", [128, G * NS * 2 * 2 * 128], bf16, kind="ExternalInput")
    b_d = nc.dram_tensor("b", [128, G * NS * 2], f32, kind="ExternalInput")
    s_d = nc.dram_tensor("s", [128, G * 2], f32, kind="ExternalInput")
    y_d = nc.dram_tensor("y", [G * 2 * 128, L], f32, kind="ExternalOutput")

    rem = L % 1024
    big_bufs = 3 if rem else 4

    with tile.TileContext(nc) as tc:
        with (
            tc.tile_pool(name="const", bufs=1) as constp,
            tc.tile_pool(name="xt", bufs=2) as xtp,
            tc.tile_pool(name="z", bufs=2) as zp,
            tc.tile_pool(name="t", bufs=2) as tp,
            tc.tile_pool(name="y", bufs=2) as yp,
            tc.tile_pool(name="psb", bufs=big_bufs, space="PSUM") as psb,
            tc.tile_pool(name="pss", bufs=2, space="PSUM") as pss,
        ):
            w_sb = constp.tile([128, G * NS * 2 * 2 * 128], bf16)
            nc.sync.dma_start(out=w_sb, in_=w_d.ap())
            b_sb = constp.tile([128, G * NS * 2], f32)
            nc.sync.dma_start(out=b_sb, in_=b_d.ap())
            s_sb = constp.tile([128, G * 2], f32)
            nc.sync.dma_start(out=s_sb, in_=s_d.ap())

            for g in range(G):
                xt_sb = xtp.tile([128, 2 * L], bf16, tag="xt")
                nc.sync.dma_start(
                    out=xt_sb, in_=xt_d.ap()[g * 128 : (g + 1) * 128]
                )
                for c in range(2):
                    zb = zp.tile([128, NS * L], f32, tag="z")
                    for s in range(NS):
                        bcol = (g * NS + s) * 2 + c
                        ps_tiles = []
                        for (c0, cl) in chunks:
                            pool = psb if cl > rem or rem == 0 else pss
                            ps_tiles.append(
                                pool.tile([128, cl], f32, tag="ps", name="ps")
                            )
                        for k in range(2):
                            blk = bcol * 2 + k
                            lhsT = w_sb[:, blk * 128 : (blk + 1) * 128]
                            for (c0, cl), ps in zip(chunks, ps_tiles):
                                for a0 in range(0, cl, 512):
                                    al = min(512, cl - a0)
                                    nc.tensor.matmul(
                                        ps[:, a0 : a0 + al],
                                        lhsT,
                                        xt_sb[:, k * L + c0 + a0 : k * L + c0 + a0 + al],
                                        start=(k == 0),
                                        stop=(k == 1),
                                    )
                        # drain PSUM via DVE with fused per-partition bias add
                        for (c0, cl), ps in zip(chunks, ps_tiles):
                            nc.vector.tensor_scalar_add(
                                zb[:, s * L + c0 : s * L + c0 + cl],
                                ps,
                                b_sb[:, bcol : bcol + 1],
                            )
                    # one big tanh over all three streams
                    t_sb = tp.tile([128, NS * L], bf16, tag="t")
                    nc.scalar.activation(t_sb, zb, Tanh)
                    ya = yp.tile([128, L], f32, tag="ya")
                    nc.vector.scalar_tensor_tensor(
                        ya,
                        t_sb[:, 0:L],
                        s_sb[:, g * 2 : g * 2 + 1],
                        t_sb[:, 2 * L : 3 * L],
                        mult,
                        add,
                    )
                    yb = yp.tile([128, L], f32, tag="yb")
                    nc.vector.scalar_tensor_tensor(
                        yb,
                        t_sb[:, L : 2 * L],
                        s_sb[:, g * 2 + 1 : g * 2 + 2],
                        ya,
                        mult,
                        add,
                    )
                    row = (g * 2 + c) * 128
                    nc.sync.dma_start(
                        out=y_d.ap()[row : row + 128], in_=yb
                    )

    nc.compile()
    return nc


def _host_route(type_embeddings, gate_w):
    """Gate on host: per-type top-2 experts + softmax weights (tiny)."""
    logits = type_embeddings.astype(np.float32) @ gate_w.astype(np.float32)
    top2 = np.argsort(-logits, axis=1, kind="stable")[:, :2]
    tv = np.take_along_axis(logits, top2, axis=1)
    e = np.exp(tv - tv.max(axis=1, keepdims=True))
    wts = e / e.sum(axis=1, keepdims=True)
    return top2, wts


def kernel(x, type_embeddings, atom_types, gate_w, expert_w, expert_b,
           shared_w, shared_b, _trace=False, _trace_kwargs=None):
    x = np.asarray(x, dtype=np.float32)
    type_embeddings = np.asarray(type_embeddings, dtype=np.float32)
    atom_types = np.asarray(atom_types)
    gate_w = np.asarray(gate_w, dtype=np.float32)
    expert_w = np.asarray(expert_w, dtype=np.float32)
    expert_b = np.asarray(expert_b, dtype=np.float32)
    shared_w = np.asarray(shared_w, dtype=np.float32)
    shared_b = np.asarray(shared_b, dtype=np.float32)

    top2, wts = _host_route(type_embeddings, gate_w)

    flat_t = atom_types.reshape(-1).astype(np.int64)
    N = flat_t.size
    order = np.argsort(flat_t, kind="stable")
    counts = np.bincount(flat_t, minlength=NTYPES)
    L = max(int(np.ceil(counts.max() / 128) * 128), 512)

    # scatter sorted atoms into per-type padded blocks [NTYPES, L, DIN]
    pos = np.repeat(np.arange(NTYPES, dtype=np.int64) * L, counts)
    pos += np.arange(N, dtype=np.int64) - np.repeat(
        np.concatenate([[0], np.cumsum(counts)[:-1]]), counts
    )
    padded = np.zeros((NTYPES * L, DIN), np.float32)
    padded[pos] = x.reshape(N, DIN)[order]

    # XT layout per core: [G*128, 2*L] bf16 with [g*128+p, k*L+a]
    xt_all = (
        padded.reshape(NTYPES, L, 2, 128)  # [t, a, k, p]
        .transpose(0, 2, 3, 1)  # [t, k, p, a]
        .astype(BF16)
    )

    # per-(type, stream) weight/bias selection
    w_sel = np.empty((NTYPES, NS, DIN, DOUT), np.float32)
    b_sel = np.empty((NTYPES, NS, DOUT), np.float32)
    for t in range(NTYPES):
        w_sel[t, 0] = expert_w[top2[t, 0]]
        w_sel[t, 1] = expert_w[top2[t, 1]]
        w_sel[t, 2] = shared_w[0]
        b_sel[t, 0] = expert_b[top2[t, 0]]
        b_sel[t, 1] = expert_b[top2[t, 1]]
        b_sel[t, 2] = shared_b[0]

    in_maps = []
    for core in range(N_CORES):
        tsl = slice(core * G, (core + 1) * G)
        xt = np.ascontiguousarray(
            xt_all[tsl].transpose(0, 2, 1, 3)  # [g, p, k, a]
        ).reshape(G * 128, 2 * L)

        wb = (
            w_sel[tsl]  # [G, NS, din, dout]
            .reshape(G, NS, 2, 128, 2, 128)  # [g, s, k, p, c, q]
            .transpose(3, 0, 1, 4, 2, 5)  # [p, g, s, c, k, q]
            .reshape(128, G * NS * 2 * 2 * 128)
            .astype(BF16)
        )
        bb = (
            b_sel[tsl]  # [G, NS, dout]
            .reshape(G, NS, 2, 128)  # [g, s, c, p]
            .transpose(3, 0, 1, 2)  # [p, g, s, c]
            .reshape(128, G * NS * 2)
            .astype(np.float32)
        )
        sb_arr = np.broadcast_to(
            wts[tsl].reshape(1, G * 2), (128, G * 2)
        ).astype(np.float32)
        in_maps.append(
            {"xt": np.ascontiguousarray(xt),
             "w": np.ascontiguousarray(wb),
             "b": np.ascontiguousarray(bb),
             "s": np.ascontiguousarray(sb_arr)}
        )

    if L not in _compiled_cache:
        _compiled_cache[L] = _build_nc(L)
    nc = _compiled_cache[L]

    kwargs = {}
    if _trace:
        kwargs["trace"] = True
        kwargs.update(_trace_kwargs or {})
    res = run_bass_kernel_spmd(nc, in_maps, core_ids=list(range(N_CORES)), **kwargs)

    # reassemble: results[core]["y"] is [G*2*128, L] = [g, c, p, a]
    y_all = np.stack([r["y"] for r in res.results])  # [core, G*2*128, L]
    y_all = y_all.reshape(NTYPES, 2, 128, L).transpose(0, 3, 1, 2)  # [t, a, c, p]
    out = np.zeros((N, DOUT), np.float32)
    out[order] = y_all.reshape(NTYPES * L, DOUT)[pos]
    out = out.reshape(NB, NLOC, DOUT)

    if _trace:
        return out, res
    return out


# revision 8
# speedup vs baseline: 1.7339x; 1.6524x over previous
"""Trainium2 Bass kernel for the MoE routing layer (nn_MoELayer_20358144983731).

Strategy
--------
Routing depends only on the atom's type (32 types), and with top-2-of-8
routing each atom needs exactly 3 expert MLPs (2 routed + 1 shared) instead
of the reference's dense 9.  The gate is tiny, so it is computed on the host;
atoms are sorted by type, each type block padded to a fixed length L, and the
32 type blocks are distributed 4-per-core across the 8 NeuronCores.  The per
type routing weights (w0, w1) are scalars shared by every atom of the type,
so the whole device program is data-driven (weights / biases / scales arrive
as per-core input tensors) and a single SPMD program runs on all 8 cores.

Per core (g = 4 type groups):
    y[g] = w0*tanh(X W0 + b0) + w1*tanh(X W1 + b1) + tanh(X Ws + bs)
computed transposed (z.T = [dout, atoms]) so the dout-dim bias lands on
partitions (free in ACT) and matmuls keep W stationary.  Matmuls run in
bf16 with fp32 PSUM accumulation; tanh on the scalar engine; the 3-stream
combine is two fused scalar_tensor_tensor ops on the vector engine.
"""

import sys

import numpy as np

try:
    import concourse  # noqa: F401
except ImportError:  # grading container path
    sys.path.insert(0, "/opt/trn_rl_repo")

import ml_dtypes

import concourse.bacc as bacc
import concourse.mybir as mybir
import concourse.tile as tile
from concourse.bass_utils import run_bass_kernel_spmd

NB, NLOC = 4, 16384
DIN, DOUT = 256, 256
NTYPES = 32
N_CORES = 8
G = NTYPES // N_CORES  # type groups per core = 4
NS = 3  # streams: routed expert 0, routed expert 1, shared expert
BF16 = ml_dtypes.bfloat16

_compiled_cache = {}


def _build_nc(L):
    """Build + compile the SPMD Tile kernel for per-type capacity L."""
    f32 = mybir.dt.float32
    bf16 = mybir.dt.bfloat16
    Tanh = mybir.ActivationFunctionType.Tanh
    mult = mybir.AluOpType.mult
    add = mybir.AluOpType.add

    # atom chunks: one big chunk (3 PSUM banks, double-buffered) + one tail
    # chunk (2 banks, single-buffered) -> 8 banks total, ACT drains PSUM
    if L <= 1536:
        chunks = [(0, L)]
    else:
        chunks = [(0, 1536), (1536, L - 1536)]
    assert L - 1536 <= 1024

    nc = bacc.Bacc("TRN2", target_bir_lowering=False, debug=False)
    xt_d = nc.dram_tensor("xt", [G * 128, 2 * L], bf16, kind="ExternalInput")
    w_d = nc.dram_tensor("w", [128, G * NS * 2 * 2 * 128], bf16, kind="ExternalInput")
    b_d = nc.dram_tensor("b", [128, G * NS * 2], f32, kind="ExternalInput")
    s_d = nc.dram_tensor("s", [128, G * 2], f32, kind="ExternalInput")
    y_d = nc.dram_tensor("y", [G * 2 * 128, L], f32, kind="ExternalOutput")

    with tile.TileContext(nc) as tc:
        with (
            tc.tile_pool(name="const", bufs=1) as constp,
            tc.tile_pool(name="xt", bufs=2) as xtp,
            tc.tile_pool(name="t", bufs=2) as tp,
            tc.tile_pool(name="y", bufs=2) as yp,
            tc.tile_pool(name="psb", bufs=2, space="PSUM") as psb,
            tc.tile_pool(name="pss", bufs=1, space="PSUM") as pss,
        ):
            w_sb = constp.tile([128, G * NS * 2 * 2 * 128], bf16)
            nc.sync.dma_start(out=w_sb, in_=w_d.ap())
            b_sb = constp.tile([128, G * NS * 2], f32)
            nc.sync.dma_start(out=b_sb, in_=b_d.ap())
            s_sb = constp.tile([128, G * 2], f32)
            nc.sync.dma_start(out=s_sb, in_=s_d.ap())

            for g in range(G):
                xt_sb = xtp.tile([128, 2 * L], bf16, tag="xt")
                nc.sync.dma_start(
                    out=xt_sb, in_=xt_d.ap()[g * 128 : (g + 1) * 128]
                )
                for c in range(2):
                    t_sb = tp.tile([128, NS * L], bf16, tag="t")
                    for s in range(NS):
                        bcol = (g * NS + s) * 2 + c
                        for ci, (c0, cl) in enumerate(chunks):
                            pool = psb if ci == 0 else pss
                            ps = pool.tile([128, cl], f32, tag="ps", name="ps")
                            for k in range(2):
                                blk = bcol * 2 + k
                                lhsT = w_sb[:, blk * 128 : (blk + 1) * 128]
                                for a0 in range(0, cl, 512):
                                    al = min(512, cl - a0)
                                    nc.tensor.matmul(
                                        ps[:, a0 : a0 + al],
                                        lhsT,
                                        xt_sb[:, k * L + c0 + a0 : k * L + c0 + a0 + al],
                                        start=(k == 0),
                                        stop=(k == 1),
                                    )
                            # tanh + per-partition bias, PSUM -> SBUF (bf16)
                            nc.scalar.activation(
                                t_sb[:, s * L + c0 : s * L + c0 + cl],
                                ps,
                                Tanh,
                                bias=b_sb[:, bcol : bcol + 1],
                                scale=1.0,
                            )
                    ya = yp.tile([128, L], f32, tag="ya")
                    nc.vector.scalar_tensor_tensor(
                        ya,
                        t_sb[:, 0:L],
                        s_sb[:, g * 2 : g * 2 + 1],
                        t_sb[:, 2 * L : 3 * L],
                        mult,
                        add,
                    )
                    yb = yp.tile([128, L], f32, tag="yb")
                    nc.vector.scalar_tensor_tensor(
                        yb,
                        t_sb[:, L : 2 * L],
                        s_sb[:, g * 2 + 1 : g * 2 + 2],
                        ya,
                        mult,
                        add,
                    )
                    row = (g * 2 + c) * 128
                    nc.sync.dma_start(
                        out=y_d.ap()[row : row + 128], in_=yb
                    )

    nc.compile()
    return nc


def _host_route(type_embeddings, gate_w):
    """Gate on host: per-type top-2 experts + softmax weights (tiny)."""
    logits = type_embeddings.astype(np.float32) @ gate_w.astype(np.float32)
    top2 = np.argsort(-logits, axis=1, kind="stable")[:, :2]
    tv = np.take_along_axis(logits, top2, axis=1)
    e = np.exp(tv - tv.max(axis=1, keepdims=True))
    wts = e / e.sum(axis=1, keepdims=True)
    return top2, wts


def kernel(x, type_embeddings, atom_types, gate_w, expert_w, expert_b,
           shared_w, shared_b, _trace=False, _trace_kwargs=None):
    x = np.asarray(x, dtype=np.float32)
    type_embeddings = np.asarray(type_embeddings, dtype=np.float32)
    atom_types = np.asarray(atom_types)
    gate_w = np.asarray(gate_w, dtype=np.float32)
    expert_w = np.asarray(expert_w, dtype=np.float32)
    expert_b = np.asarray(expert_b, dtype=np.float32)
    shared_w = np.asarray(shared_w, dtype=np.float32)
    shared_b = np.asarray(shared_b, dtype=np.float32)

    top2, wts = _host_route(type_embeddings, gate_w)

    flat_t = atom_types.reshape(-1).astype(np.int64)
    N = flat_t.size
    order = np.argsort(flat_t, kind="stable")
    counts = np.bincount(flat_t, minlength=NTYPES)
    L = max(int(np.ceil(counts.max() / 128) * 128), 512)

    # scatter sorted atoms into per-type padded blocks [NTYPES, L, DIN]
    pos = np.repeat(np.arange(NTYPES, dtype=np.int64) * L, counts)
    pos += np.arange(N, dtype=np.int64) - np.repeat(
        np.concatenate([[0], np.cumsum(counts)[:-1]]), counts
    )
    padded = np.zeros((NTYPES * L, DIN), np.float32)
    padded[pos] = x.reshape(N, DIN)[order]

    # XT layout per core: [G*128, 2*L] bf16 with [g*128+p, k*L+a]
    xt_all = (
        padded.reshape(NTYPES, L, 2, 128)  # [t, a, k, p]
        .transpose(0, 2, 3, 1)  # [t, k, p, a]
        .astype(BF16)
    )

    # per-(type, stream) weight/bias selection
    w_sel = np.empty((NTYPES, NS, DIN, DOUT), np.float32)
    b_sel = np.empty((NTYPES, NS, DOUT), np.float32)
    for t in range(NTYPES):
        w_sel[t, 0] = expert_w[top2[t, 0]]
        w_sel[t, 1] = expert_w[top2[t, 1]]
        w_sel[t, 2] = shared_w[0]
        b_sel[t, 0] = expert_b[top2[t, 0]]
        b_sel[t, 1] = expert_b[top2[t, 1]]
        b_sel[t, 2] = shared_b[0]

    in_maps = []
    for core in range(N_CORES):
        tsl = slice(core * G, (core + 1) * G)
        xt = np.ascontiguousarray(
            xt_all[tsl].transpose(0, 2, 1, 3)  # [g, p, k, a]
        ).reshape(G * 128, 2 * L)

        wb = (
            w_sel[tsl]  # [G, NS, din, dout]
            .reshape(G, NS, 2, 128, 2, 128)  # [g, s, k, p, c, q]
            .transpose(3, 0, 1, 4, 2, 5)  # [p, g, s, c, k, q]
            .reshape(128, G * NS * 2 * 2 * 128)
            .astype(BF16)
        )
        bb = (
            b_sel[tsl]  # [G, NS, dout]
            .reshape(G, NS, 2, 128)  # [g, s, c, p]
            .transpose(3, 0, 1, 2)  # [p, g, s, c]
            .reshape(128, G * NS * 2)
            .astype(np.float32)
        )
        sb_arr = np.broadcast_to(
            wts[tsl].reshape(1, G * 2), (128, G * 2)
        ).astype(np.float32)
        in_maps.append(
            {"xt": np.ascontiguousarray(xt),
             "w": np.ascontiguousarray(wb),
             "b": np.ascontiguousarray(bb),
             "s": np.ascontiguousarray(sb_arr)}
        )

    if L not in _compiled_cache:
        _compiled_cache[L] = _build_nc(L)
    nc = _compiled_cache[L]

    kwargs = {}
    if _trace:
        kwargs["trace"] = True
        kwargs.update(_trace_kwargs or {})
    res = run_bass_kernel_spmd(nc, in_maps, core_ids=list(range(N_CORES)), **kwargs)

    # reassemble: results[core]["y"] is [G*2*128, L] = [g, c, p, a]
    y_all = np.stack([r["y"] for r in res.results])  # [core, G*2*128, L]
    y_all = y_all.reshape(NTYPES, 2, 128, L).transpose(0, 3, 1, 2)  # [t, a, c, p]
    out = np.zeros((N, DOUT), np.float32)
    out[order] = y_all.reshape(NTYPES * L, DOUT)[pos]
    out = out.reshape(NB, NLOC, DOUT)

    if _trace:
        return out, res
    return out


# revision 13
# speedup vs baseline: 1.7804x; 1.0268x over previous
"""Trainium2 Bass kernel for the MoE routing layer (nn_MoELayer_20358144983731).

Strategy
--------
Routing depends only on the atom's type (32 types), and with top-2-of-8
routing each atom needs exactly 3 expert MLPs (2 routed + 1 shared) instead
of the reference's dense 9.  The gate is tiny, so it is computed on the host;
atoms are sorted by type, each type block padded to a fixed length L, and the
32 type blocks are distributed 4-per-core across the 8 NeuronCores.  The per
type routing weights (w0, w1) are scalars shared by every atom of the type,
so the whole device program is data-driven (weights / biases / scales arrive
as per-core input tensors) and a single SPMD program runs on all 8 cores.

Per core (g = 4 type groups):
    y[g] = w0*tanh(X W0 + b0) + w1*tanh(X W1 + b1) + tanh(X Ws + bs)
computed transposed (z.T = [dout, atoms]) so the dout-dim bias lands on
partitions (free in ACT) and matmuls keep W stationary.  Matmuls run in
bf16 with fp32 PSUM accumulation; tanh on the scalar engine; the 3-stream
combine is two fused scalar_tensor_tensor ops on the vector engine.
"""

import sys

import numpy as np

try:
    import concourse  # noqa: F401
except ImportError:  # grading container path
    sys.path.insert(0, "/opt/trn_rl_repo")

import ml_dtypes

import concourse.bacc as bacc
import concourse.mybir as mybir
import concourse.tile as tile
from concourse.bass_utils import run_bass_kernel_spmd

NB, NLOC = 4, 16384
DIN, DOUT = 256, 256
NTYPES = 32
N_CORES = 8
G = NTYPES // N_CORES  # type groups per core = 4
NS = 3  # streams: routed expert 0, routed expert 1, shared expert
BF16 = ml_dtypes.bfloat16

_compiled_cache = {}


def _build_nc(L):
    """Build + compile the SPMD Tile kernel for per-type capacity L."""
    f32 = mybir.dt.float32
    bf16 = mybir.dt.bfloat16
    Tanh = mybir.ActivationFunctionType.Tanh
    mult = mybir.AluOpType.mult
    add = mybir.AluOpType.add

    # atom chunks: one big chunk (3 PSUM banks, double-buffered) + one tail
    # chunk (2 banks, single-buffered) -> 8 banks total, ACT drains PSUM
    if L <= 1536:
        chunks = [(0, L)]
    else:
        chunks = [(0, 1536), (1536, L - 1536)]
    assert L - 1536 <= 1024

    nc = bacc.Bacc("TRN2", target_bir_lowering=False, debug=False)
    xt_d = nc.dram_tensor("xt", [G * 128, 2 * L], bf16, kind="ExternalInput")
    w_d = nc.dram_tensor("w", [128, G * NS * 2 * 2 * 128], bf16, kind="ExternalInput")
    b_d = nc.dram_tensor("b", [128, G * NS * 2], f32, kind="ExternalInput")
    s_d = nc.dram_tensor("s", [128, G * 2], f32, kind="ExternalInput")
    y_d = nc.dram_tensor("y", [G * 2 * 128, L], f32, kind="ExternalOutput")

    with tile.TileContext(nc) as tc:
        with (
            tc.tile_pool(name="const", bufs=1) as constp,
            tc.tile_pool(name="xt", bufs=2) as xtp,
            tc.tile_pool(name="t", bufs=2) as tp,
            tc.tile_pool(name="y", bufs=2) as yp,
            tc.tile_pool(name="psb", bufs=2, space="PSUM") as psb,
            tc.tile_pool(name="pss", bufs=1, space="PSUM") as pss,
        ):
            WG = NS * 2 * 2 * 128  # weight columns per group

            # first group's inputs land first so PE starts ASAP
            xt0 = [xtp.tile([128, L], bf16, tag=f"xt{k}", name=f"xt{k}") for k in range(2)]
            for k in range(2):
                nc.sync.dma_start(
                    out=xt0[k], in_=xt_d.ap()[0:128, k * L : (k + 1) * L]
                )
            w_gs = []
            for g in range(G):
                w_g = constp.tile([128, WG], bf16, name=f"w{g}")
                w_gs.append(w_g)
            nc.sync.dma_start(out=w_gs[0], in_=w_d.ap()[:, 0:WG])
            b_sb = constp.tile([128, G * NS * 2], f32)
            nc.sync.dma_start(out=b_sb, in_=b_d.ap())
            s_sb = constp.tile([128, G * 2], f32)
            nc.sync.dma_start(out=s_sb, in_=s_d.ap())
            for g in range(1, G):
                nc.sync.dma_start(
                    out=w_gs[g], in_=w_d.ap()[:, g * WG : (g + 1) * WG]
                )

            for g in range(G):
                if g == 0:
                    xt_sb = xt0
                else:
                    xt_sb = [
                        xtp.tile([128, L], bf16, tag=f"xt{k}", name=f"xt{k}")
                        for k in range(2)
                    ]
                    for k in range(2):
                        nc.sync.dma_start(
                            out=xt_sb[k],
                            in_=xt_d.ap()[
                                g * 128 : (g + 1) * 128, k * L : (k + 1) * L
                            ],
                        )
                for c in range(2):
                    t_sb = tp.tile([128, NS * L], bf16, tag="t")
                    for s in range(NS):
                        bcol = (g * NS + s) * 2 + c
                        for ci, (c0, cl) in enumerate(chunks):
                            pool = psb if ci == 0 else pss
                            ps = pool.tile([128, cl], f32, tag="ps", name="ps")
                            for k in range(2):
                                blk = (s * 2 + c) * 2 + k
                                lhsT = w_gs[g][:, blk * 128 : (blk + 1) * 128]
                                for a0 in range(0, cl, 512):
                                    al = min(512, cl - a0)
                                    nc.tensor.matmul(
                                        ps[:, a0 : a0 + al],
                                        lhsT,
                                        xt_sb[k][:, c0 + a0 : c0 + a0 + al],
                                        start=(k == 0),
                                        stop=(k == 1),
                                    )
                            # tanh + per-partition bias, PSUM -> SBUF (bf16)
                            nc.scalar.activation(
                                t_sb[:, s * L + c0 : s * L + c0 + cl],
                                ps,
                                Tanh,
                                bias=b_sb[:, bcol : bcol + 1],
                                scale=1.0,
                            )
                    row = (g * 2 + c) * 128
                    half = (L // 2 + 511) // 512 * 512
                    yb = yp.tile([128, L], f32, tag="yb")
                    for h0, h1 in ((0, half), (half, L)):
                        ya = yp.tile([128, half], f32, tag="ya", name="ya")
                        nc.vector.scalar_tensor_tensor(
                            ya[:, : h1 - h0],
                            t_sb[:, h0:h1],
                            s_sb[:, g * 2 : g * 2 + 1],
                            t_sb[:, 2 * L + h0 : 2 * L + h1],
                            mult,
                            add,
                        )
                        nc.vector.scalar_tensor_tensor(
                            yb[:, h0:h1],
                            t_sb[:, L + h0 : L + h1],
                            s_sb[:, g * 2 + 1 : g * 2 + 2],
                            ya[:, : h1 - h0],
                            mult,
                            add,
                        )
                        nc.sync.dma_start(
                            out=y_d.ap()[row : row + 128, h0:h1],
                            in_=yb[:, h0:h1],
                        )

    nc.compile()
    return nc


def _host_route(type_embeddings, gate_w):
    """Gate on host: per-type top-2 experts + softmax weights (tiny)."""
    logits = type_embeddings.astype(np.float32) @ gate_w.astype(np.float32)
    top2 = np.argsort(-logits, axis=1, kind="stable")[:, :2]
    tv = np.take_along_axis(logits, top2, axis=1)
    e = np.exp(tv - tv.max(axis=1, keepdims=True))
    wts = e / e.sum(axis=1, keepdims=True)
    return top2, wts


def kernel(x, type_embeddings, atom_types, gate_w, expert_w, expert_b,
           shared_w, shared_b, _trace=False, _trace_kwargs=None):
    x = np.asarray(x, dtype=np.float32)
    type_embeddings = np.asarray(type_embeddings, dtype=np.float32)
    atom_types = np.asarray(atom_types)
    gate_w = np.asarray(gate_w, dtype=np.float32)
    expert_w = np.asarray(expert_w, dtype=np.float32)
    expert_b = np.asarray(expert_b, dtype=np.float32)
    shared_w = np.asarray(shared_w, dtype=np.float32)
    shared_b = np.asarray(shared_b, dtype=np.float32)

    top2, wts = _host_route(type_embeddings, gate_w)

    flat_t = atom_types.reshape(-1).astype(np.int64)
    N = flat_t.size
    order = np.argsort(flat_t, kind="stable")
    counts = np.bincount(flat_t, minlength=NTYPES)
    L = max(int(np.ceil(counts.max() / 128) * 128), 512)

    # scatter sorted atoms into per-type padded blocks [NTYPES, L, DIN]
    pos = np.repeat(np.arange(NTYPES, dtype=np.int64) * L, counts)
    pos += np.arange(N, dtype=np.int64) - np.repeat(
        np.concatenate([[0], np.cumsum(counts)[:-1]]), counts
    )
    padded = np.zeros((NTYPES * L, DIN), np.float32)
    padded[pos] = x.reshape(N, DIN)[order]

    # XT layout per core: [G*128, 2*L] bf16 with [g*128+p, k*L+a]
    xt_all = (
        padded.reshape(NTYPES, L, 2, 128)  # [t, a, k, p]
        .transpose(0, 2, 3, 1)  # [t, k, p, a]
        .astype(BF16)
    )

    # per-(type, stream) weight/bias selection
    w_sel = np.empty((NTYPES, NS, DIN, DOUT), np.float32)
    b_sel = np.empty((NTYPES, NS, DOUT), np.float32)
    for t in range(NTYPES):
        w_sel[t, 0] = expert_w[top2[t, 0]]
        w_sel[t, 1] = expert_w[top2[t, 1]]
        w_sel[t, 2] = shared_w[0]
        b_sel[t, 0] = expert_b[top2[t, 0]]
        b_sel[t, 1] = expert_b[top2[t, 1]]
        b_sel[t, 2] = shared_b[0]

    in_maps = []
    for core in range(N_CORES):
        tsl = slice(core * G, (core + 1) * G)
        xt = np.ascontiguousarray(
            xt_all[tsl].transpose(0, 2, 1, 3)  # [g, p, k, a]
        ).reshape(G * 128, 2 * L)

        wb = (
            w_sel[tsl]  # [G, NS, din, dout]
            .reshape(G, NS, 2, 128, 2, 128)  # [g, s, k, p, c, q]
            .transpose(3, 0, 1, 4, 2, 5)  # [p, g, s, c, k, q]
            .reshape(128, G * NS * 2 * 2 * 128)
            .astype(BF16)
        )
        bb = (
            b_sel[tsl]  # [G, NS, dout]
            .reshape(G, NS, 2, 128)  # [g, s, c, p]
            .transpose(3, 0, 1, 2)  # [p, g, s, c]
            .reshape(128, G * NS * 2)
            .astype(np.float32)
        )
        sb_arr = np.broadcast_to(
            wts[tsl].reshape(1, G * 2), (128, G * 2)
        ).astype(np.float32)
        in_maps.append(
            {"xt": np.ascontiguousarray(xt),
             "w": np.ascontiguousarray(wb),
             "b": np.ascontiguousarray(bb),
             "s": np.ascontiguousarray(sb_arr)}
        )

    if L not in _compiled_cache:
        _compiled_cache[L] = _build_nc(L)
    nc = _compiled_cache[L]

    kwargs = {}
    if _trace:
        kwargs["trace"] = True
        kwargs.update(_trace_kwargs or {})
    res = run_bass_kernel_spmd(nc, in_maps, core_ids=list(range(N_CORES)), **kwargs)

    # reassemble: results[core]["y"] is [G*2*128, L] = [g, c, p, a]
    y_all = np.stack([r["y"] for r in res.results])  # [core, G*2*128, L]
    y_all = y_all.reshape(NTYPES, 2, 128, L).transpose(0, 3, 1, 2)  # [t, a, c, p]
    out = np.zeros((N, DOUT), np.float32)
    out[order] = y_all.reshape(NTYPES * L, DOUT)[pos]
    out = out.reshape(NB, NLOC, DOUT)

    if _trace:
        return out, res
    return out


# revision 15
# speedup vs baseline: 1.8480x; 1.0380x over previous
"""Trainium2 Bass kernel for the MoE routing layer (nn_MoELayer_20358144983731).

Strategy
--------
Routing depends only on the atom's type (32 types), and with top-2-of-8
routing each atom needs exactly 3 expert MLPs (2 routed + 1 shared) instead
of the reference's dense 9.  The gate is tiny, so it is computed on the host;
atoms are sorted by type, each type block padded to a fixed length L, and the
32 type blocks are distributed 4-per-core across the 8 NeuronCores.  The per
type routing weights (w0, w1) are scalars shared by every atom of the type,
so the whole device program is data-driven (weights / biases / scales arrive
as per-core input tensors) and a single SPMD program runs on all 8 cores.

Per core (g = 4 type groups):
    y[g] = w0*tanh(X W0 + b0) + w1*tanh(X W1 + b1) + tanh(X Ws + bs)
computed transposed (z.T = [dout, atoms]) so the dout-dim bias lands on
partitions (free in ACT) and matmuls keep W stationary.  Matmuls run in
bf16 with fp32 PSUM accumulation; tanh on the scalar engine; the 3-stream
combine is two fused scalar_tensor_tensor ops on the vector engine.
"""

import sys

import numpy as np

try:
    import concourse  # noqa: F401
except ImportError:  # grading container path
    sys.path.insert(0, "/opt/trn_rl_repo")

import ml_dtypes

import concourse.bacc as bacc
import concourse.mybir as mybir
import concourse.tile as tile
from concourse.bass_utils import run_bass_kernel_spmd

NB, NLOC = 4, 16384
DIN, DOUT = 256, 256
NTYPES = 32
N_CORES = 8
G = NTYPES // N_CORES  # type groups per core = 4
NS = 3  # streams: routed expert 0, routed expert 1, shared expert
BF16 = ml_dtypes.bfloat16

_compiled_cache = {}


def _build_nc(L):
    """Build + compile the SPMD Tile kernel for per-type capacity L."""
    f32 = mybir.dt.float32
    bf16 = mybir.dt.bfloat16
    Tanh = mybir.ActivationFunctionType.Tanh
    mult = mybir.AluOpType.mult
    add = mybir.AluOpType.add

    # atom chunks: one big chunk (3 PSUM banks, double-buffered) + one tail
    # chunk (2 banks, single-buffered) -> 8 banks total, ACT drains PSUM
    if L <= 1536:
        chunks = [(0, L)]
    else:
        chunks = [(0, 1536), (1536, L - 1536)]
    assert L - 1536 <= 1024

    nc = bacc.Bacc("TRN2", target_bir_lowering=False, debug=False)
    xt_d = nc.dram_tensor("xt", [G * 128, 2 * L], bf16, kind="ExternalInput")
    w_d = nc.dram_tensor("w", [128, G * NS * 2 * 2 * 128], bf16, kind="ExternalInput")
    b_d = nc.dram_tensor("b", [128, G * NS * 2], f32, kind="ExternalInput")
    s_d = nc.dram_tensor("s", [128, G * 2], f32, kind="ExternalInput")
    y_d = nc.dram_tensor("y", [G * 2 * 128, L], f32, kind="ExternalOutput")

    with tile.TileContext(nc) as tc:
        with (
            tc.tile_pool(name="const", bufs=1) as constp,
            tc.tile_pool(name="xt", bufs=2) as xtp,
            tc.tile_pool(name="t", bufs=2) as tp,
            tc.tile_pool(name="y", bufs=2) as yp,
            tc.tile_pool(name="psb", bufs=2, space="PSUM") as psb,
            tc.tile_pool(name="pss", bufs=1, space="PSUM") as pss,
        ):
            WG = NS * 2 * 2 * 128  # weight columns per group

            # first group's inputs land first so PE starts ASAP
            w_gs = [constp.tile([128, WG], bf16, name=f"w{g}") for g in range(G)]
            nc.sync.dma_start(out=w_gs[0], in_=w_d.ap()[:, 0:WG])
            xt0 = [xtp.tile([128, L], bf16, tag=f"xt{k}", name=f"xt{k}") for k in range(2)]
            for k in range(2):
                nc.sync.dma_start(
                    out=xt0[k], in_=xt_d.ap()[0:128, k * L : (k + 1) * L]
                )
            # non-critical constants go on the scalar HWDGE queue (parallel issue)
            b_sb = constp.tile([128, G * NS * 2], f32)
            nc.scalar.dma_start(out=b_sb, in_=b_d.ap())
            s_sb = constp.tile([128, G * 2], f32)
            nc.scalar.dma_start(out=s_sb, in_=s_d.ap())
            for g in range(1, G):
                nc.scalar.dma_start(
                    out=w_gs[g], in_=w_d.ap()[:, g * WG : (g + 1) * WG]
                )

            for g in range(G):
                if g == 0:
                    xt_sb = xt0
                else:
                    xt_sb = [
                        xtp.tile([128, L], bf16, tag=f"xt{k}", name=f"xt{k}")
                        for k in range(2)
                    ]
                    for k in range(2):
                        nc.sync.dma_start(
                            out=xt_sb[k],
                            in_=xt_d.ap()[
                                g * 128 : (g + 1) * 128, k * L : (k + 1) * L
                            ],
                        )
                for c in range(2):
                    t_sb = tp.tile([128, NS * L], bf16, tag="t")
                    # shared stream (s=2) first: the combines need t2+t0 before
                    # t1, so the tail combine only waits on the last stream
                    for s in (2, 0, 1):
                        bcol = (g * NS + s) * 2 + c
                        for ci, (c0, cl) in enumerate(chunks):
                            pool = psb if ci == 0 else pss
                            ps = pool.tile([128, cl], f32, tag="ps", name="ps")
                            for k in range(2):
                                blk = (s * 2 + c) * 2 + k
                                lhsT = w_gs[g][:, blk * 128 : (blk + 1) * 128]
                                for a0 in range(0, cl, 512):
                                    al = min(512, cl - a0)
                                    nc.tensor.matmul(
                                        ps[:, a0 : a0 + al],
                                        lhsT,
                                        xt_sb[k][:, c0 + a0 : c0 + a0 + al],
                                        start=(k == 0),
                                        stop=(k == 1),
                                    )
                            # tanh + per-partition bias, PSUM -> SBUF (bf16)
                            nc.scalar.activation(
                                t_sb[:, s * L + c0 : s * L + c0 + cl],
                                ps,
                                Tanh,
                                bias=b_sb[:, bcol : bcol + 1],
                                scale=1.0,
                            )
                    row = (g * 2 + c) * 128
                    half = (L // 2 + 511) // 512 * 512
                    yb = yp.tile([128, L], f32, tag="yb")
                    for h0, h1 in ((0, half), (half, L)):
                        ya = yp.tile([128, half], f32, tag="ya", name="ya")
                        nc.vector.scalar_tensor_tensor(
                            ya[:, : h1 - h0],
                            t_sb[:, h0:h1],
                            s_sb[:, g * 2 : g * 2 + 1],
                            t_sb[:, 2 * L + h0 : 2 * L + h1],
                            mult,
                            add,
                        )
                        nc.vector.scalar_tensor_tensor(
                            yb[:, h0:h1],
                            t_sb[:, L + h0 : L + h1],
                            s_sb[:, g * 2 + 1 : g * 2 + 2],
                            ya[:, : h1 - h0],
                            mult,
                            add,
                        )
                        nc.sync.dma_start(
                            out=y_d.ap()[row : row + 128, h0:h1],
                            in_=yb[:, h0:h1],
                        )

    nc.compile()
    return nc


def _host_route(type_embeddings, gate_w):
    """Gate on host: per-type top-2 experts + softmax weights (tiny)."""
    logits = type_embeddings.astype(np.float32) @ gate_w.astype(np.float32)
    top2 = np.argsort(-logits, axis=1, kind="stable")[:, :2]
    tv = np.take_along_axis(logits, top2, axis=1)
    e = np.exp(tv - tv.max(axis=1, keepdims=True))
    wts = e / e.sum(axis=1, keepdims=True)
    return top2, wts


def kernel(x, type_embeddings, atom_types, gate_w, expert_w, expert_b,
           shared_w, shared_b, _trace=False, _trace_kwargs=None):
    x = np.asarray(x, dtype=np.float32)
    type_embeddings = np.asarray(type_embeddings, dtype=np.float32)
    atom_types = np.asarray(atom_types)
    gate_w = np.asarray(gate_w, dtype=np.float32)
    expert_w = np.asarray(expert_w, dtype=np.float32)
    expert_b = np.asarray(expert_b, dtype=np.float32)
    shared_w = np.asarray(shared_w, dtype=np.float32)
    shared_b = np.asarray(shared_b, dtype=np.float32)

    top2, wts = _host_route(type_embeddings, gate_w)

    flat_t = atom_types.reshape(-1).astype(np.int64)
    N = flat_t.size
    order = np.argsort(flat_t, kind="stable")
    counts = np.bincount(flat_t, minlength=NTYPES)
    L = max(int(np.ceil(counts.max() / 128) * 128), 512)

    # scatter sorted atoms into per-type padded blocks [NTYPES, L, DIN]
    pos = np.repeat(np.arange(NTYPES, dtype=np.int64) * L, counts)
    pos += np.arange(N, dtype=np.int64) - np.repeat(
        np.concatenate([[0], np.cumsum(counts)[:-1]]), counts
    )
    padded = np.zeros((NTYPES * L, DIN), np.float32)
    padded[pos] = x.reshape(N, DIN)[order]

    # XT layout per core: [G*128, 2*L] bf16 with [g*128+p, k*L+a]
    xt_all = (
        padded.reshape(NTYPES, L, 2, 128)  # [t, a, k, p]
        .transpose(0, 2, 3, 1)  # [t, k, p, a]
        .astype(BF16)
    )

    # per-(type, stream) weight/bias selection
    w_sel = np.empty((NTYPES, NS, DIN, DOUT), np.float32)
    b_sel = np.empty((NTYPES, NS, DOUT), np.float32)
    for t in range(NTYPES):
        w_sel[t, 0] = expert_w[top2[t, 0]]
        w_sel[t, 1] = expert_w[top2[t, 1]]
        w_sel[t, 2] = shared_w[0]
        b_sel[t, 0] = expert_b[top2[t, 0]]
        b_sel[t, 1] = expert_b[top2[t, 1]]
        b_sel[t, 2] = shared_b[0]

    in_maps = []
    for core in range(N_CORES):
        tsl = slice(core * G, (core + 1) * G)
        xt = np.ascontiguousarray(
            xt_all[tsl].transpose(0, 2, 1, 3)  # [g, p, k, a]
        ).reshape(G * 128, 2 * L)

        wb = (
            w_sel[tsl]  # [G, NS, din, dout]
            .reshape(G, NS, 2, 128, 2, 128)  # [g, s, k, p, c, q]
            .transpose(3, 0, 1, 4, 2, 5)  # [p, g, s, c, k, q]
            .reshape(128, G * NS * 2 * 2 * 128)
            .astype(BF16)
        )
        bb = (
            b_sel[tsl]  # [G, NS, dout]
            .reshape(G, NS, 2, 128)  # [g, s, c, p]
            .transpose(3, 0, 1, 2)  # [p, g, s, c]
            .reshape(128, G * NS * 2)
            .astype(np.float32)
        )
        sb_arr = np.broadcast_to(
            wts[tsl].reshape(1, G * 2), (128, G * 2)
        ).astype(np.float32)
        in_maps.append(
            {"xt": np.ascontiguousarray(xt),
             "w": np.ascontiguousarray(wb),
             "b": np.ascontiguousarray(bb),
             "s": np.ascontiguousarray(sb_arr)}
        )

    if L not in _compiled_cache:
        _compiled_cache[L] = _build_nc(L)
    nc = _compiled_cache[L]

    kwargs = {}
    if _trace:
        kwargs["trace"] = True
        kwargs.update(_trace_kwargs or {})
    res = run_bass_kernel_spmd(nc, in_maps, core_ids=list(range(N_CORES)), **kwargs)

    # reassemble: results[core]["y"] is [G*2*128, L] = [g, c, p, a]
    y_all = np.stack([r["y"] for r in res.results])  # [core, G*2*128, L]
    y_all = y_all.reshape(NTYPES, 2, 128, L).transpose(0, 3, 1, 2)  # [t, a, c, p]
    out = np.zeros((N, DOUT), np.float32)
    out[order] = y_all.reshape(NTYPES * L, DOUT)[pos]
    out = out.reshape(NB, NLOC, DOUT)

    if _trace:
        return out, res
    return out
